# revision 13
# baseline (speedup 1.0000x reference)
"""Per-channel EMA (first-order linear recurrence along time) on 8 TRN2 cores.

  y[b, c, 0] = x[b, c, 0]
  y[b, c, t] = (1 - alpha[c]) * y[b, c, t-1] + alpha[c] * x[b, c, t]

Strategy (v4: radix-2 scan + PE offload + fp16 IO + packed host layout)
  - Data-parallel over batch: B=32 -> 4 batches per core, alpha replicated.
  - Per core: 16 tiles of [128 channels (partitions), 2048 time (free)].
  - The DVE tensor_tensor_scan runs at ~2.1 cycles/element regardless of
    dtype, so a full-tile scan costs ~4.4us and 16 of them (~86us) dominated
    the v1 kernel. Here the recurrence is decimated by 2:
        even outputs:  z_m = y_{2m} = d^2 * z_{m-1} + u_m,
                       u_m = a*d*x_{2m-1} + a*x_{2m}   (u_0 = x_0)
        odd outputs:   y_{2m+1} = d * z_m + a * x_{2m+1}
    The DVE scans only the 1024 even columns (~2.3us/tile); u and the odd
    reconstruction are diagonal matmuls on the otherwise-idle Tensor engine
    (per-channel scale = diag weight matrix, fp16, PSUM f32 accumulation):
        u    = diag(a*d) @ x_odd<<1 + diag(a) @ x_even  (+ diag(d) @ x_0 on
               column 0, making u_0 = (a+d)*x_0 = x_0 exactly)
        y_od = diag(d) @ z + diag(a) @ x_odd
    The ACT engine copies the odd PSUM back to SBUF. Diag weights (fp16) and
    d^2 (fp32) are precomputed on host from alpha - alpha is a kernel input.
  - Host packs each core's x into [128, 16*2048] fp16: tile-major, each tile
    block holding its 1024 even time steps then its 1024 odd ones. Every
    device access is contiguous (strided fp16 costs the PE ~3x) and every
    load/store is one contiguous-per-partition DMA (~144 descriptors, so the
    ~1.2us fixed descriptor-gen cost amortizes over whole tile groups).
  - HBM IO is fp16: halves the 32 MiB/core round trip to 16 MiB (~47us
    roofline at 358 GB/s/core). The scan state stays fp32 internally with
    d^2 in fp32, so the recurrence does not accumulate quantization error
    (|d|<1 contraction; measured rel err ~5e-4, gate is 2e-2).
  - Queue discipline: the SP HWDGE queue only wakes ~8.7us in (engine-init
    barriers), the ACT ring ~2.5us - so weights, tile 0 (two chained
    half-chunks) and the (1,2) pair load ride the ACT ring, and the SP queue
    carries pair loads (3,4)..(13,14) plus 15. PE is emitted with a one-tile
    skew (BCA_{n+1} before DE_n) so it pipelines around the scan; stores
    ride SWDGE on the idle GpSimd queue except tiles 14/15 which use the
    ACT HWDGE ring to dodge the slow SWDGE tail drain.
  - The last tile runs as four chained quarter-chunks with a quarter-
    interleaved y block ([ev_q | od_q] x4), so each quarter finishes with a
    single contiguous ring store and the drain point moves as early as the
    dependency chain allows.
"""

import numpy as np

import concourse.bass as bass
import concourse.bacc as bacc
import concourse.mybir as mybir
from concourse.tile import TileContext
from concourse.bass_utils import run_bass_kernel_spmd

B, C, L = 32, 512, 2048
N_CORES = 8
B_SH = B // N_CORES  # 4 batches per core
P = 128              # SBUF partitions
N_CB = C // P        # 4 channel blocks
N_TILES = B_SH * N_CB
LH = L // 2          # 1024 scan columns per tile
LQ = LH // 4         # 256-column quarters for the last tile

_F32 = mybir.dt.float32
_F16 = mybir.dt.float16

mult = mybir.AluOpType.mult
add = mybir.AluOpType.add

LAST = N_TILES - 1


def build_nc() -> bass.Bass:
    # Bacc (not raw Bass): its compile() runs generate_event_semaphores,
    # which splits multi-sem waits — TRN2 allows at most one wait command
    # per instruction, and Tile freely emits several.
    nc = bacc.Bacc()
    x = nc.dram_tensor("x", [P, N_TILES * L], _F16, kind="ExternalInput")
    # w[p, (cb*3+j)*P + m]: diag weight blocks, j=0: diag(a), 1: diag(a*d),
    # 2: diag(d) for channel block cb (built on host, already in SBUF layout)
    w = nc.dram_tensor("w", [P, N_CB * 3 * P], _F16, kind="ExternalInput")
    d2 = nc.dram_tensor("d2", [1, C], _F32, kind="ExternalInput")
    y = nc.dram_tensor("y", [P, N_TILES * L], _F16, kind="ExternalOutput")

    with TileContext(nc) as tc:
        with (
            tc.tile_pool(name="xp", bufs=4) as xp,
            tc.tile_pool(name="yp", bufs=6) as yp,
            tc.tile_pool(name="cp", bufs=1) as cp,
            tc.tile_pool(name="up", bufs=2, space="PSUM") as up,
            tc.tile_pool(name="wp", bufs=2, space="PSUM") as wp,
        ):
            # consts + early loads ride the ACT HWDGE ring (it wakes ~6us
            # before the SP queue)
            wt = cp.tile([P, N_CB * 3 * P], _F16, tag="wt", name="wt")
            nc.scalar.dma_start(out=wt, in_=w[:, :])
            d2t = cp.tile([P, N_CB], _F32, tag="d2t", name="d2t")
            nc.scalar.dma_start(
                out=d2t, in_=d2[0].rearrange("(j p) -> p j", j=N_CB)
            )
            # warm-up ACT op: pulls the activation-table load off the first
            # odd-copy's critical path (depends only on the tiny d2 load)
            warm = cp.tile([P, N_CB], _F32, tag="warm", name="warm")
            nc.scalar.mul(warm, d2t, 1.0)

            def W(cb, j):
                o = (cb * 3 + j) * P
                return wt[:, o : o + P]

            xv = [None] * N_TILES  # per-tile [P, L] x views (ev | od)
            yv = [None] * N_TILES  # per-tile [P, L] y views

            def load_group(n0, n1, engine):
                """One DMA for tiles [n0, n1); contiguous in the packed
                layout. Returns the group tile."""
                k = n1 - n0
                g = xp.tile([P, k * L], _F16, tag=f"x{k}", name=f"xg{n0}")
                engine.dma_start(out=g, in_=x[:, n0 * L : n1 * L])
                for i in range(k):
                    xv[n0 + i] = g[:, i * L : (i + 1) * L]

            def alloc_y(n):
                yv[n] = yp.tile([P, L], _F16, tag="y", name=f"yt{n}")

            tiles_u = [None] * N_TILES
            tiles_w = [None] * N_TILES

            def alloc_psum(n):
                tiles_u[n] = up.tile([P, LH], _F32, tag="u", name="u")
                tiles_w[n] = wp.tile([P, LH], _F32, tag="w", name="wv")

            # last-tile y block is quarter-interleaved: [ev_q0|od_q0|...]
            def zcols(n, lo, hi):
                if n == LAST:
                    q = lo // LQ
                    assert hi - lo == LQ and lo % LQ == 0
                    return yv[n][:, q * 2 * LQ : q * 2 * LQ + LQ]
                return yv[n][:, lo:hi]

            def ocols(n, lo, hi):
                if n == LAST:
                    q = lo // LQ
                    assert hi - lo == LQ and lo % LQ == 0
                    return yv[n][:, q * 2 * LQ + LQ : (q + 1) * 2 * LQ]
                return yv[n][:, LH + lo : LH + hi]

            def emit_bca(n, lo, hi):
                """u[lo:hi] = diag(a)@x_ev[lo:hi] + diag(ad)@x_od[lo-1:hi-1]
                (+ diag(d)@x_0 on column 0), emitted as <=512-col matmuls."""
                xt, u = xv[n], tiles_u[n]
                cb = n // B_SH
                Wa, Wad, Wd = W(cb, 0), W(cb, 1), W(cb, 2)
                for s in range(lo, hi, 512):
                    e = min(s + 512, hi)
                    nc.tensor.matmul(
                        out=u[:, s:e], lhsT=Wa, rhs=xt[:, s:e],
                        start=True, stop=False,
                    )
                for s in range(lo, hi, 512):
                    e = min(s + 512, hi)
                    s2 = max(s, 1)  # u_0 has no x_{-1} term
                    # stop on the final writer of each PSUM region: regions
                    # containing column 0 are finished by the A-matmul below
                    nc.tensor.matmul(
                        out=u[:, s2:e], lhsT=Wad,
                        rhs=xt[:, LH + s2 - 1 : LH + e - 1],
                        start=False, stop=(s > 0),
                    )
                if lo == 0:
                    nc.tensor.matmul(
                        out=u[:, 0:1], lhsT=Wd, rhs=xt[:, 0:1],
                        start=False, stop=True,
                    )

            def emit_scan(n, lo, hi):
                cb = n // B_SH
                if lo == 0:
                    init = 0.0
                elif n == LAST:
                    init = zcols(n, lo - LQ, lo)[:, LQ - 1 : LQ]
                else:
                    init = yv[n][:, lo - 1 : lo]
                nc.vector.tensor_tensor_scan(
                    out=zcols(n, lo, hi),
                    data0=d2t[:, cb : cb + 1].broadcast_to([P, hi - lo]),
                    data1=tiles_u[n][:, lo:hi],
                    initial=init,
                    op0=mult,
                    op1=add,
                )

            def emit_de(n, lo, hi):
                """wv[lo:hi] = diag(d) @ z[lo:hi] + diag(a) @ x_od[lo:hi]"""
                xt, wv = xv[n], tiles_w[n]
                cb = n // B_SH
                Wa, Wd = W(cb, 0), W(cb, 2)
                for s in range(lo, hi, 512):
                    e = min(s + 512, hi)
                    nc.tensor.matmul(
                        out=wv[:, s:e], lhsT=Wd, rhs=zcols(n, s, e),
                        start=True, stop=False,
                    )
                for s in range(lo, hi, 512):
                    e = min(s + 512, hi)
                    nc.tensor.matmul(
                        out=wv[:, s:e], lhsT=Wa,
                        rhs=xt[:, LH + s : LH + e],
                        start=False, stop=True,
                    )

            def emit_copy(n, lo, hi):
                nc.scalar.copy(ocols(n, lo, hi), tiles_w[n][:, lo:hi])

            # tile 0: two chained half-chunks on the ACT ring for a fast
            # pipeline start (four 128 KiB pieces)
            g0 = xp.tile([P, L], _F16, tag="x1", name="xg0")
            xv[0] = g0
            for c in range(2):
                lo, hi = c * (LH // 2), (c + 1) * (LH // 2)
                nc.scalar.dma_start(out=g0[:, lo:hi], in_=x[:, lo:hi])
                nc.scalar.dma_start(
                    out=g0[:, LH + lo : LH + hi],
                    in_=x[:, LH + lo : LH + hi],
                )
            # the (1,2) pair load also rides the ring: the SP queue wakes
            # too late to feed scan_1 on time
            load_group(1, 3, nc.scalar)
            # remaining loads on the SP queue as pairs
            for n0 in range(3, LAST, 2):
                load_group(n0, n0 + 2, nc.sync)
            load_group(LAST, LAST + 1, nc.sync)

            alloc_y(0)
            alloc_psum(0)
            for c in range(2):
                lo, hi = c * (LH // 2), (c + 1) * (LH // 2)
                emit_bca(0, lo, hi)
                emit_scan(0, lo, hi)

            for n in range(1, LAST):
                alloc_y(n)
                alloc_psum(n)
                emit_bca(n, 0, LH)
                emit_de(n - 1, 0, LH)
                emit_scan(n, 0, LH)
                emit_copy(n - 1, 0, LH)
                # store tile n-1
                m = n - 1
                if m >= N_TILES - 2:
                    nc.scalar.dma_start(
                        out=y[:, m * L : (m + 1) * L], in_=yv[m]
                    )
                else:
                    nc.gpsimd.dma_start(
                        out=y[:, m * L : (m + 1) * L], in_=yv[m]
                    )

            # last tile: four chained quarter-chunks; each quarter's
            # [ev|od] block stores contiguously on the ACT ring
            alloc_y(LAST)
            alloc_psum(LAST)
            emit_bca(LAST, 0, LH)
            emit_de(LAST - 1, 0, LH)
            for k in range(4):
                emit_scan(LAST, k * LQ, (k + 1) * LQ)
            emit_copy(LAST - 1, 0, LH)
            nc.scalar.dma_start(
                out=y[:, (LAST - 1) * L : LAST * L], in_=yv[LAST - 1]
            )
            for k in range(4):
                lo, hi = k * LQ, (k + 1) * LQ
                emit_de(LAST, lo, hi)
                emit_copy(LAST, lo, hi)
                nc.scalar.dma_start(
                    out=y[:, LAST * L + 2 * lo : LAST * L + 2 * hi],
                    in_=yv[LAST][:, 2 * lo : 2 * hi],
                )

    nc.compile()
    return nc


def _host_consts(alpha: np.ndarray):
    """Diag weight blocks (fp16, SBUF layout) + d^2 (fp32) from alpha."""
    a = alpha[0].astype(np.float64)  # [C]
    d = 1.0 - a
    # fp16 diag entries; d16 = 1 - a16 in fp16 arithmetic so the u_0 column
    # fixup (a16 + d16) lands as close to exactly 1 as fp16 allows
    a16 = a.astype(np.float16)
    d16 = (np.float16(1.0) - a16).astype(np.float16)
    ad16 = (a16 * d16).astype(np.float16)
    w = np.zeros((P, N_CB * 3 * P), dtype=np.float16)
    idx = np.arange(P)
    for cb in range(N_CB):
        s = slice(cb * P, (cb + 1) * P)
        for j, v in enumerate((a16[s], ad16[s], d16[s])):
            w[idx, (cb * 3 + j) * P + idx] = v
    d2 = (d * d).astype(np.float32)[None, :]  # [1, C]
    return w, d2


def _pack_core(xc: np.ndarray) -> np.ndarray:
    """[B_SH, C, L] -> [P, N_TILES*L] fp16: tile n = (cb*B_SH + b), block
    layout [evens | odds]."""
    x5 = xc.reshape(B_SH, N_CB, P, LH, 2)        # b, cb, p, m, parity
    x5 = x5.transpose(2, 1, 0, 4, 3)             # p, cb, b, parity, m
    return np.ascontiguousarray(x5, dtype=np.float16).reshape(P, N_TILES * L)


def _unpack_core(yc: np.ndarray) -> np.ndarray:
    """Inverse of _pack_core (+ the last tile's quarter-interleaved block)."""
    yc = yc.reshape(P, N_TILES, L).astype(np.float32)
    std = np.empty((P, N_TILES, 2, LH), dtype=np.float32)
    std[:, :, 0, :] = yc[:, :, 0:LH]
    std[:, :, 1, :] = yc[:, :, LH:L]
    lastq = yc[:, LAST].reshape(P, 4, 2, LQ)     # p, quarter, parity, m
    std[:, LAST, 0] = lastq[:, :, 0].reshape(P, LH)
    std[:, LAST, 1] = lastq[:, :, 1].reshape(P, LH)
    # std: p, (cb b), parity, m  ->  b, cb*P+p, 2m+parity
    y5 = std.reshape(P, N_CB, B_SH, 2, LH).transpose(2, 1, 0, 4, 3)
    return np.ascontiguousarray(y5).reshape(B_SH, C, L)


_cached_nc = None


def _get_nc() -> bass.Bass:
    global _cached_nc
    if _cached_nc is None:
        _cached_nc = build_nc()
    return _cached_nc


def run(x: np.ndarray, alpha: np.ndarray, nc=None, **spmd_kwargs):
    """Full host path: prep inputs, run on 8 cores, reassemble output.
    Returns (y, BassKernelResults)."""
    assert x.shape == (B, C, L) and alpha.shape == (1, C)
    x = np.asarray(x, dtype=np.float32)
    alpha = np.ascontiguousarray(alpha, dtype=np.float32)
    w, d2 = _host_consts(alpha)
    if nc is None:
        nc = _get_nc()
    in_maps = [
        {"x": _pack_core(x[c * B_SH : (c + 1) * B_SH]), "w": w, "d2": d2}
        for c in range(N_CORES)
    ]
    res = run_bass_kernel_spmd(nc, in_maps, list(range(N_CORES)), **spmd_kwargs)
    y = np.concatenate([_unpack_core(r["y"]) for r in res.results], axis=0)
    return y, res


def kernel(x: np.ndarray, alpha: np.ndarray) -> np.ndarray:
    return run(x, alpha)[0]


# revision 16
# speedup vs baseline: 1.0114x; 1.0114x over previous
"""Per-channel EMA (first-order linear recurrence along time) on 8 TRN2 cores.

  y[b, c, 0] = x[b, c, 0]
  y[b, c, t] = (1 - alpha[c]) * y[b, c, t-1] + alpha[c] * x[b, c, t]

Strategy (v5: radix-2 scan + PE offload, bf16 IO, packed layout, 2+2 queues)
  - Data-parallel over batch: B=32 -> 4 batches per core, alpha replicated.
  - Per core: 16 tiles of [128 channels (partitions), 2048 time (free)].
  - The DVE tensor_tensor_scan runs at ~2.1 cycles/element regardless of
    dtype, so a full-tile scan costs ~4.4us and 16 of them (~86us) dominated
    the v1 kernel. Here the recurrence is decimated by 2:
        even outputs:  z_m = y_{2m} = d^2 * z_{m-1} + u_m,
                       u_m = a*d*x_{2m-1} + a*x_{2m}   (u_0 = x_0)
        odd outputs:   y_{2m+1} = d * z_m + a * x_{2m+1}
    The DVE scans only the 1024 even columns (~2.3us/tile); u and the odd
    reconstruction are diagonal matmuls on the otherwise-idle Tensor engine
    (per-channel scale = diag weight matrix, PSUM f32 accumulation):
        u    = diag(a*d) @ x_odd<<1 + diag(a) @ x_even  (+ diag(d) @ x_0 on
               column 0, making u_0 = (a+d)*x_0 = x_0 exactly)
        y_od = diag(d) @ z + diag(a) @ x_odd
    The ACT engine copies the odd PSUM back to SBUF. Diag weights and d^2
    (fp32) are precomputed on host from alpha - alpha is a kernel input.
  - Everything HBM-facing is bf16: halves the 32 MiB/core round trip AND
    runs the PE at its fast rate (fp16 matmuls measured ~2x slower). The
    scan state stays fp32 internally with d^2 in fp32, so the recurrence
    does not accumulate quantization error (|d|<1 contraction; measured rel
    err ~2e-3, gate is 2e-2).
  - Host packs each core's x into [128, 16*2048] bf16: tile-major, each tile
    block holding its 1024 even time steps then its 1024 odd ones. Every
    device access is contiguous and every load is one contiguous-per-
    partition DMA (~144 descriptors).
  - Queue discipline (each HWDGE trigger costs ~0.65us of its engine queue,
    and a queue serializes its own transfers, so saturating HBM needs two
    load queues + two store queues):
      * the PE queue wakes earliest (~5us): tile-0's first half + x1 ride
        nc.tensor.dma_start as its first instructions, and odd tiles'
        loads interleave into the PE queue with 2-tile lookahead;
      * even tiles' loads ride the SP (sync) queue;
      * weights/d2 + tile-0's second half ride the ACT ring ahead of the
        copies; the ACT-table warm-up op sits after them;
      * stores alternate SWDGE-on-GpSimd / ACT ring; the last three tiles
        all use the ring (SWDGE completion lags ~10us and would push out
        the kernel drain).
  - PE is emitted with a one-tile skew (BCA_{n+1} before DE_n) so it
    pipelines around the scan; tile 0 runs as two chained half-chunks, the
    last tile as four chained quarter-chunks whose [ev|od] blocks store
    contiguously (quarter-interleaved y block), pulling the drain forward.
"""

import numpy as np
import ml_dtypes

import concourse.bass as bass
import concourse.bacc as bacc
import concourse.mybir as mybir
from concourse.tile import TileContext
from concourse.bass_utils import run_bass_kernel_spmd

B, C, L = 32, 512, 2048
N_CORES = 8
B_SH = B // N_CORES  # 4 batches per core
P = 128              # SBUF partitions
N_CB = C // P        # 4 channel blocks
N_TILES = B_SH * N_CB
LH = L // 2          # 1024 scan columns per tile
LQ = LH // 4         # 256-column quarters for the last tile

_F32 = mybir.dt.float32
_BF16 = mybir.dt.bfloat16
_NP_BF16 = ml_dtypes.bfloat16

mult = mybir.AluOpType.mult
add = mybir.AluOpType.add

LAST = N_TILES - 1


def build_nc() -> bass.Bass:
    # Bacc (not raw Bass): its compile() runs generate_event_semaphores,
    # which splits multi-sem waits — TRN2 allows at most one wait command
    # per instruction, and Tile freely emits several.
    nc = bacc.Bacc()
    x = nc.dram_tensor("x", [P, N_TILES * L], _BF16, kind="ExternalInput")
    # w[p, (cb*3+j)*P + m]: diag weight blocks, j=0: diag(a), 1: diag(a*d),
    # 2: diag(d) for channel block cb (built on host, already in SBUF layout)
    w = nc.dram_tensor("w", [P, N_CB * 3 * P], _BF16, kind="ExternalInput")
    d2 = nc.dram_tensor("d2", [1, C], _F32, kind="ExternalInput")
    y = nc.dram_tensor("y", [P, N_TILES * L], _BF16, kind="ExternalOutput")

    with TileContext(nc) as tc:
        with (
            tc.tile_pool(name="xp", bufs=5) as xp,
            tc.tile_pool(name="yp", bufs=6) as yp,
            tc.tile_pool(name="cp", bufs=1) as cp,
            tc.tile_pool(name="up", bufs=2, space="PSUM") as up,
            tc.tile_pool(name="wp", bufs=2, space="PSUM") as wp,
        ):
            xv = [None] * N_TILES  # per-tile [P, L] x views (ev | od)
            yv = [None] * N_TILES  # per-tile [P, L] y views
            tiles_u = [None] * N_TILES
            tiles_w = [None] * N_TILES

            def alloc(n):
                yv[n] = yp.tile([P, L], _BF16, tag="y", name=f"yt{n}")
                tiles_u[n] = up.tile([P, LH], _F32, tag="u", name="u")
                tiles_w[n] = wp.tile([P, LH], _F32, tag="w", name="wv")

            def load(n, engine, k=1):
                g = xp.tile([P, k * L], _BF16, tag=f"x{k}", name=f"xt{n}")
                engine.dma_start(out=g, in_=x[:, n * L : (n + k) * L])
                for i in range(k):
                    xv[n + i] = g[:, i * L : (i + 1) * L]

            # ---- startup: the ACT ring wakes ~2us before the SP queue ----
            # ACT ring: tile-0 first half, weights, d2, tile-0 second half,
            # x1, x2, then the table warm
            g0 = xp.tile([P, L], _BF16, tag="x1", name="xt0")
            xv[0] = g0
            nc.scalar.dma_start(out=g0[:, 0:512], in_=x[:, 0:512])
            nc.scalar.dma_start(
                out=g0[:, LH : LH + 512], in_=x[:, LH : LH + 512]
            )
            wt = cp.tile([P, N_CB * 3 * P], _BF16, tag="wt", name="wt")
            nc.scalar.dma_start(out=wt, in_=w[:, :])
            d2t = cp.tile([P, N_CB], _F32, tag="d2t", name="d2t")
            nc.scalar.dma_start(
                out=d2t, in_=d2[0].rearrange("(j p) -> p j", j=N_CB)
            )
            nc.scalar.dma_start(out=g0[:, 512:LH], in_=x[:, 512:LH])
            nc.scalar.dma_start(out=g0[:, LH + 512 : L], in_=x[:, LH + 512 : L])
            load(1, nc.scalar)
            load(2, nc.scalar)
            # warm-up ACT op: pulls the activation-table load off the first
            # odd-copy's critical path
            warm = cp.tile([P, N_CB], _F32, tag="warm", name="warm")
            nc.scalar.mul(warm, d2t, 1.0)
            # SP queue: singles 3..9, then pairs to compress the stream tail
            for n in range(3, 10):
                load(n, nc.sync)
            load(10, nc.sync, k=2)
            load(12, nc.sync, k=2)
            load(14, nc.sync, k=2)

            def W(cb, j):
                o = (cb * 3 + j) * P
                return wt[:, o : o + P]

            # last-tile y block is quarter-interleaved: [ev_q0|od_q0|...]
            def zcols(n, lo, hi):
                if n == LAST:
                    q = lo // LQ
                    assert hi - lo == LQ and lo % LQ == 0
                    return yv[n][:, q * 2 * LQ : q * 2 * LQ + LQ]
                return yv[n][:, lo:hi]

            def ocols(n, lo, hi):
                if n == LAST:
                    q = lo // LQ
                    assert hi - lo == LQ and lo % LQ == 0
                    return yv[n][:, q * 2 * LQ + LQ : (q + 1) * 2 * LQ]
                return yv[n][:, LH + lo : LH + hi]

            def emit_bca(n, lo, hi):
                """u[lo:hi] = diag(a)@x_ev[lo:hi] + diag(ad)@x_od[lo-1:hi-1]
                (+ diag(d)@x_0 on column 0), emitted as <=512-col matmuls."""
                xt, u = xv[n], tiles_u[n]
                cb = n // B_SH
                Wa, Wad, Wd = W(cb, 0), W(cb, 1), W(cb, 2)
                for s in range(lo, hi, 512):
                    e = min(s + 512, hi)
                    nc.tensor.matmul(
                        out=u[:, s:e], lhsT=Wa, rhs=xt[:, s:e],
                        start=True, stop=False,
                    )
                for s in range(lo, hi, 512):
                    e = min(s + 512, hi)
                    s2 = max(s, 1)  # u_0 has no x_{-1} term
                    # stop on the final writer of each PSUM region: regions
                    # containing column 0 are finished by the A-matmul below
                    nc.tensor.matmul(
                        out=u[:, s2:e], lhsT=Wad,
                        rhs=xt[:, LH + s2 - 1 : LH + e - 1],
                        start=False, stop=(s > 0),
                    )
                if lo == 0:
                    nc.tensor.matmul(
                        out=u[:, 0:1], lhsT=Wd, rhs=xt[:, 0:1],
                        start=False, stop=True,
                    )

            def emit_scan(n, lo, hi):
                cb = n // B_SH
                if lo == 0:
                    init = 0.0
                elif n == LAST:
                    init = zcols(n, lo - LQ, lo)[:, LQ - 1 : LQ]
                else:
                    init = yv[n][:, lo - 1 : lo]
                nc.vector.tensor_tensor_scan(
                    out=zcols(n, lo, hi),
                    data0=d2t[:, cb : cb + 1].broadcast_to([P, hi - lo]),
                    data1=tiles_u[n][:, lo:hi],
                    initial=init,
                    op0=mult,
                    op1=add,
                )

            def emit_de(n, lo, hi):
                """wv[lo:hi] = diag(d) @ z[lo:hi] + diag(a) @ x_od[lo:hi]"""
                xt, wv = xv[n], tiles_w[n]
                cb = n // B_SH
                Wa, Wd = W(cb, 0), W(cb, 2)
                for s in range(lo, hi, 512):
                    e = min(s + 512, hi)
                    nc.tensor.matmul(
                        out=wv[:, s:e], lhsT=Wd, rhs=zcols(n, s, e),
                        start=True, stop=False,
                    )
                for s in range(lo, hi, 512):
                    e = min(s + 512, hi)
                    nc.tensor.matmul(
                        out=wv[:, s:e], lhsT=Wa,
                        rhs=xt[:, LH + s : LH + e],
                        start=False, stop=True,
                    )

            def emit_copy(n, lo, hi):
                nc.scalar.copy(ocols(n, lo, hi), tiles_w[n][:, lo:hi])

            def emit_store(m):
                # odd tiles + the last three ride the ACT ring; SWDGE
                # completion lag would stall the drain and pool recycling
                ring = (m % 2 == 1) or m >= N_TILES - 3
                dma = nc.scalar.dma_start if ring else nc.gpsimd.dma_start
                dma(out=y[:, m * L : (m + 1) * L], in_=yv[m])

            # tile 0: two chained half-chunks
            alloc(0)
            for c in range(2):
                lo, hi = c * (LH // 2), (c + 1) * (LH // 2)
                emit_bca(0, lo, hi)
                emit_scan(0, lo, hi)

            for n in range(1, LAST):
                alloc(n)
                emit_bca(n, 0, LH)
                emit_de(n - 1, 0, LH)
                emit_scan(n, 0, LH)
                emit_copy(n - 1, 0, LH)
                emit_store(n - 1)

            # last tile: four chained quarter-chunks; each quarter's
            # [ev|od] block stores contiguously on the ACT ring
            alloc(LAST)
            emit_bca(LAST, 0, LH)
            emit_de(LAST - 1, 0, LH)
            for k in range(4):
                emit_scan(LAST, k * LQ, (k + 1) * LQ)
            emit_copy(LAST - 1, 0, LH)
            emit_store(LAST - 1)
            for k in range(4):
                lo, hi = k * LQ, (k + 1) * LQ
                emit_de(LAST, lo, hi)
                emit_copy(LAST, lo, hi)
                nc.scalar.dma_start(
                    out=y[:, LAST * L + 2 * lo : LAST * L + 2 * hi],
                    in_=yv[LAST][:, 2 * lo : 2 * hi],
                )

    nc.compile()
    return nc


def _host_consts(alpha: np.ndarray):
    """Diag weight blocks (bf16, SBUF layout) + d^2 (fp32) from alpha."""
    a = alpha[0].astype(np.float64)  # [C]
    d = 1.0 - a
    # bf16 diag entries; d16 = 1 - a16 in bf16 arithmetic so the u_0 column
    # fixup (a16 + d16) lands as close to exactly 1 as bf16 allows
    a16 = a.astype(_NP_BF16)
    d16 = (_NP_BF16(1.0) - a16).astype(_NP_BF16)
    ad16 = (a16 * d16).astype(_NP_BF16)
    w = np.zeros((P, N_CB * 3 * P), dtype=_NP_BF16)
    idx = np.arange(P)
    for cb in range(N_CB):
        s = slice(cb * P, (cb + 1) * P)
        for j, v in enumerate((a16[s], ad16[s], d16[s])):
            w[idx, (cb * 3 + j) * P + idx] = v
    d2 = (d * d).astype(np.float32)[None, :]  # [1, C]
    return w, d2


def _pack_core(xc: np.ndarray) -> np.ndarray:
    """[B_SH, C, L] -> [P, N_TILES*L] bf16: tile n = (cb*B_SH + b), block
    layout [evens | odds]."""
    x5 = xc.reshape(B_SH, N_CB, P, LH, 2)        # b, cb, p, m, parity
    x5 = x5.transpose(2, 1, 0, 4, 3)             # p, cb, b, parity, m
    return np.ascontiguousarray(
        x5.astype(_NP_BF16)
    ).reshape(P, N_TILES * L)


def _unpack_core(yc: np.ndarray) -> np.ndarray:
    """Inverse of _pack_core (+ the last tile's quarter-interleaved block)."""
    yc = yc.reshape(P, N_TILES, L).astype(np.float32)
    std = np.empty((P, N_TILES, 2, LH), dtype=np.float32)
    std[:, :, 0, :] = yc[:, :, 0:LH]
    std[:, :, 1, :] = yc[:, :, LH:L]
    lastq = yc[:, LAST].reshape(P, 4, 2, LQ)     # p, quarter, parity, m
    std[:, LAST, 0] = lastq[:, :, 0].reshape(P, LH)
    std[:, LAST, 1] = lastq[:, :, 1].reshape(P, LH)
    # std: p, (cb b), parity, m  ->  b, cb*P+p, 2m+parity
    y5 = std.reshape(P, N_CB, B_SH, 2, LH).transpose(2, 1, 0, 4, 3)
    return np.ascontiguousarray(y5).reshape(B_SH, C, L)


_cached_nc = None


def _get_nc() -> bass.Bass:
    global _cached_nc
    if _cached_nc is None:
        _cached_nc = build_nc()
    return _cached_nc


def run(x: np.ndarray, alpha: np.ndarray, nc=None, **spmd_kwargs):
    """Full host path: prep inputs, run on 8 cores, reassemble output.
    Returns (y, BassKernelResults)."""
    assert x.shape == (B, C, L) and alpha.shape == (1, C)
    x = np.asarray(x, dtype=np.float32)
    alpha = np.ascontiguousarray(alpha, dtype=np.float32)
    w, d2 = _host_consts(alpha)
    if nc is None:
        nc = _get_nc()
    in_maps = [
        {"x": _pack_core(x[c * B_SH : (c + 1) * B_SH]), "w": w, "d2": d2}
        for c in range(N_CORES)
    ]
    res = run_bass_kernel_spmd(nc, in_maps, list(range(N_CORES)), **spmd_kwargs)
    y = np.concatenate([_unpack_core(r["y"]) for r in res.results], axis=0)
    return y, res


def kernel(x: np.ndarray, alpha: np.ndarray) -> np.ndarray:
    return run(x, alpha)[0]


# revision 17
# speedup vs baseline: 1.0193x; 1.0078x over previous
"""Per-channel EMA (first-order linear recurrence along time) on 8 TRN2 cores.

  y[b, c, 0] = x[b, c, 0]
  y[b, c, t] = (1 - alpha[c]) * y[b, c, t-1] + alpha[c] * x[b, c, t]

Strategy (v5: radix-2 scan + PE offload, bf16 IO, packed layout, 2+2 queues)
  - Data-parallel over batch: B=32 -> 4 batches per core, alpha replicated.
  - Per core: 16 tiles of [128 channels (partitions), 2048 time (free)].
  - The DVE tensor_tensor_scan runs at ~2.1 cycles/element regardless of
    dtype, so a full-tile scan costs ~4.4us and 16 of them (~86us) dominated
    the v1 kernel. Here the recurrence is decimated by 2:
        even outputs:  z_m = y_{2m} = d^2 * z_{m-1} + u_m,
                       u_m = a*d*x_{2m-1} + a*x_{2m}   (u_0 = x_0)
        odd outputs:   y_{2m+1} = d * z_m + a * x_{2m+1}
    The DVE scans only the 1024 even columns (~2.3us/tile); u and the odd
    reconstruction are diagonal matmuls on the otherwise-idle Tensor engine
    (per-channel scale = diag weight matrix, PSUM f32 accumulation):
        u    = diag(a*d) @ x_odd<<1 + diag(a) @ x_even  (+ diag(d) @ x_0 on
               column 0, making u_0 = (a+d)*x_0 = x_0 exactly)
        y_od = diag(d) @ z + diag(a) @ x_odd
    The ACT engine copies the odd PSUM back to SBUF. Diag weights and d^2
    (fp32) are precomputed on host from alpha - alpha is a kernel input.
  - Everything HBM-facing is bf16: halves the 32 MiB/core round trip AND
    runs the PE at its fast rate (fp16 matmuls measured ~2x slower). The
    scan state stays fp32 internally with d^2 in fp32, so the recurrence
    does not accumulate quantization error (|d|<1 contraction; measured rel
    err ~2e-3, gate is 2e-2).
  - Host packs each core's x into [128, 16*2048] bf16: tile-major, each tile
    block holding its 1024 even time steps then its 1024 odd ones. Every
    device access is contiguous and every load is one contiguous-per-
    partition DMA (~144 descriptors).
  - Queue discipline (each HWDGE trigger costs ~0.65us of its engine queue,
    and a queue serializes its own transfers, so saturating HBM needs two
    load queues + two store queues):
      * the PE queue wakes earliest (~5us): tile-0's first half + x1 ride
        nc.tensor.dma_start as its first instructions, and odd tiles'
        loads interleave into the PE queue with 2-tile lookahead;
      * even tiles' loads ride the SP (sync) queue;
      * weights/d2 + tile-0's second half ride the ACT ring ahead of the
        copies; the ACT-table warm-up op sits after them;
      * stores alternate SWDGE-on-GpSimd / ACT ring; the last three tiles
        all use the ring (SWDGE completion lags ~10us and would push out
        the kernel drain).
  - PE is emitted with a one-tile skew (BCA_{n+1} before DE_n) so it
    pipelines around the scan; tile 0 runs as two chained half-chunks, the
    last tile as four chained quarter-chunks whose [ev|od] blocks store
    contiguously (quarter-interleaved y block), pulling the drain forward.
"""

import numpy as np
import ml_dtypes

import concourse.bass as bass
import concourse.bacc as bacc
import concourse.mybir as mybir
from concourse.tile import TileContext
from concourse.bass_utils import run_bass_kernel_spmd

B, C, L = 32, 512, 2048
N_CORES = 8
B_SH = B // N_CORES  # 4 batches per core
P = 128              # SBUF partitions
N_CB = C // P        # 4 channel blocks
N_TILES = B_SH * N_CB
LH = L // 2          # 1024 scan columns per tile
LQ = LH // 4         # 256-column quarters for the last tile

_F32 = mybir.dt.float32
_BF16 = mybir.dt.bfloat16
_NP_BF16 = ml_dtypes.bfloat16

mult = mybir.AluOpType.mult
add = mybir.AluOpType.add

LAST = N_TILES - 1


def build_nc() -> bass.Bass:
    # Bacc (not raw Bass): its compile() runs generate_event_semaphores,
    # which splits multi-sem waits — TRN2 allows at most one wait command
    # per instruction, and Tile freely emits several.
    nc = bacc.Bacc()
    x = nc.dram_tensor("x", [P, N_TILES * L], _BF16, kind="ExternalInput")
    # w[p, (cb*3+j)*P + m]: diag weight blocks, j=0: diag(a), 1: diag(a*d),
    # 2: diag(d) for channel block cb (built on host, already in SBUF layout)
    w = nc.dram_tensor("w", [P, N_CB * 3 * P], _BF16, kind="ExternalInput")
    d2 = nc.dram_tensor("d2", [P, N_CB], _F32, kind="ExternalInput")
    y = nc.dram_tensor("y", [P, N_TILES * L], _BF16, kind="ExternalOutput")

    with TileContext(nc) as tc:
        with (
            tc.tile_pool(name="xp", bufs=5) as xp,
            tc.tile_pool(name="yp", bufs=6) as yp,
            tc.tile_pool(name="cp", bufs=1) as cp,
            tc.tile_pool(name="up", bufs=2, space="PSUM") as up,
            tc.tile_pool(name="wp", bufs=2, space="PSUM") as wp,
        ):
            xv = [None] * N_TILES  # per-tile [P, L] x views (ev | od)
            yv = [None] * N_TILES  # per-tile [P, L] y views
            tiles_u = [None] * N_TILES
            tiles_w = [None] * N_TILES

            def alloc(n):
                yv[n] = yp.tile([P, L], _BF16, tag="y", name=f"yt{n}")
                tiles_u[n] = up.tile([P, LH], _F32, tag="u", name="u")
                tiles_w[n] = wp.tile([P, LH], _F32, tag="w", name="wv")

            def load(n, engine, k=1):
                g = xp.tile([P, k * L], _BF16, tag=f"x{k}", name=f"xt{n}")
                engine.dma_start(out=g, in_=x[:, n * L : (n + k) * L])
                for i in range(k):
                    xv[n + i] = g[:, i * L : (i + 1) * L]

            # ---- startup: the ACT ring wakes ~3us before the SP queue,
            # and a queue serializes its transfers - order by criticality:
            # d2 (gates scan_0), cb0 weights (gate BCA_0), tile-0 pieces,
            # remaining weights, with the ACT-table warm-up in between
            d2t = cp.tile([P, N_CB], _F32, tag="d2t", name="d2t")
            nc.scalar.dma_start(out=d2t, in_=d2[:, :])
            wt = cp.tile([P, N_CB * 3 * P], _BF16, tag="wt", name="wt")
            nc.scalar.dma_start(out=wt[:, 0 : 3 * P], in_=w[:, 0 : 3 * P])
            # warm-up ACT op: pulls the activation-table load off the first
            # odd-copy's critical path
            warm = cp.tile([P, N_CB], _F32, tag="warm", name="warm")
            nc.scalar.mul(warm, d2t, 1.0)
            g0 = xp.tile([P, L], _BF16, tag="x1", name="xt0")
            xv[0] = g0
            nc.scalar.dma_start(out=g0[:, 0:512], in_=x[:, 0:512])
            nc.scalar.dma_start(
                out=g0[:, LH : LH + 512], in_=x[:, LH : LH + 512]
            )
            nc.scalar.dma_start(out=g0[:, 512:LH], in_=x[:, 512:LH])
            nc.scalar.dma_start(out=g0[:, LH + 512 : L], in_=x[:, LH + 512 : L])
            nc.scalar.dma_start(
                out=wt[:, 3 * P : N_CB * 3 * P], in_=w[:, 3 * P : N_CB * 3 * P]
            )
            # SP queue: x1, x2, singles 3..9, then pairs to compress the tail
            load(1, nc.sync)
            load(2, nc.sync)
            for n in range(3, 10):
                load(n, nc.sync)
            load(10, nc.sync, k=2)
            load(12, nc.sync, k=2)
            load(14, nc.sync, k=2)

            def W(cb, j):
                o = (cb * 3 + j) * P
                return wt[:, o : o + P]

            # last-tile y block is quarter-interleaved: [ev_q0|od_q0|...]
            def zcols(n, lo, hi):
                if n == LAST:
                    q = lo // LQ
                    assert hi - lo == LQ and lo % LQ == 0
                    return yv[n][:, q * 2 * LQ : q * 2 * LQ + LQ]
                return yv[n][:, lo:hi]

            def ocols(n, lo, hi):
                if n == LAST:
                    q = lo // LQ
                    assert hi - lo == LQ and lo % LQ == 0
                    return yv[n][:, q * 2 * LQ + LQ : (q + 1) * 2 * LQ]
                return yv[n][:, LH + lo : LH + hi]

            def emit_bca(n, lo, hi):
                """u[lo:hi] = diag(a)@x_ev[lo:hi] + diag(ad)@x_od[lo-1:hi-1]
                (+ diag(d)@x_0 on column 0), emitted as <=512-col matmuls."""
                xt, u = xv[n], tiles_u[n]
                cb = n // B_SH
                Wa, Wad, Wd = W(cb, 0), W(cb, 1), W(cb, 2)
                for s in range(lo, hi, 512):
                    e = min(s + 512, hi)
                    nc.tensor.matmul(
                        out=u[:, s:e], lhsT=Wa, rhs=xt[:, s:e],
                        start=True, stop=False,
                    )
                for s in range(lo, hi, 512):
                    e = min(s + 512, hi)
                    s2 = max(s, 1)  # u_0 has no x_{-1} term
                    # stop on the final writer of each PSUM region: regions
                    # containing column 0 are finished by the A-matmul below
                    nc.tensor.matmul(
                        out=u[:, s2:e], lhsT=Wad,
                        rhs=xt[:, LH + s2 - 1 : LH + e - 1],
                        start=False, stop=(s > 0),
                    )
                if lo == 0:
                    nc.tensor.matmul(
                        out=u[:, 0:1], lhsT=Wd, rhs=xt[:, 0:1],
                        start=False, stop=True,
                    )

            def emit_scan(n, lo, hi):
                cb = n // B_SH
                if lo == 0:
                    init = 0.0
                elif n == LAST:
                    init = zcols(n, lo - LQ, lo)[:, LQ - 1 : LQ]
                else:
                    init = yv[n][:, lo - 1 : lo]
                nc.vector.tensor_tensor_scan(
                    out=zcols(n, lo, hi),
                    data0=d2t[:, cb : cb + 1].broadcast_to([P, hi - lo]),
                    data1=tiles_u[n][:, lo:hi],
                    initial=init,
                    op0=mult,
                    op1=add,
                )

            def emit_de(n, lo, hi):
                """wv[lo:hi] = diag(d) @ z[lo:hi] + diag(a) @ x_od[lo:hi]"""
                xt, wv = xv[n], tiles_w[n]
                cb = n // B_SH
                Wa, Wd = W(cb, 0), W(cb, 2)
                for s in range(lo, hi, 512):
                    e = min(s + 512, hi)
                    nc.tensor.matmul(
                        out=wv[:, s:e], lhsT=Wd, rhs=zcols(n, s, e),
                        start=True, stop=False,
                    )
                for s in range(lo, hi, 512):
                    e = min(s + 512, hi)
                    nc.tensor.matmul(
                        out=wv[:, s:e], lhsT=Wa,
                        rhs=xt[:, LH + s : LH + e],
                        start=False, stop=True,
                    )

            def emit_copy(n, lo, hi):
                nc.scalar.copy(ocols(n, lo, hi), tiles_w[n][:, lo:hi])

            def emit_store(m):
                # odd tiles + the last three ride the ACT ring; SWDGE
                # completion lag would stall the drain and pool recycling
                ring = (m % 2 == 1) or m >= N_TILES - 3
                dma = nc.scalar.dma_start if ring else nc.gpsimd.dma_start
                dma(out=y[:, m * L : (m + 1) * L], in_=yv[m])

            # tile 0: two chained half-chunks
            alloc(0)
            for c in range(2):
                lo, hi = c * (LH // 2), (c + 1) * (LH // 2)
                emit_bca(0, lo, hi)
                emit_scan(0, lo, hi)

            for n in range(1, LAST):
                alloc(n)
                emit_bca(n, 0, LH)
                emit_de(n - 1, 0, LH)
                emit_scan(n, 0, LH)
                emit_copy(n - 1, 0, LH)
                emit_store(n - 1)

            # last tile: four chained quarter-chunks; each quarter's
            # [ev|od] block stores contiguously on the ACT ring
            alloc(LAST)
            emit_bca(LAST, 0, LH)
            emit_de(LAST - 1, 0, LH)
            for k in range(4):
                emit_scan(LAST, k * LQ, (k + 1) * LQ)
            emit_copy(LAST - 1, 0, LH)
            emit_store(LAST - 1)
            for k in range(4):
                lo, hi = k * LQ, (k + 1) * LQ
                emit_de(LAST, lo, hi)
                emit_copy(LAST, lo, hi)
                nc.scalar.dma_start(
                    out=y[:, LAST * L + 2 * lo : LAST * L + 2 * hi],
                    in_=yv[LAST][:, 2 * lo : 2 * hi],
                )

    nc.compile()
    return nc


def _host_consts(alpha: np.ndarray):
    """Diag weight blocks (bf16, SBUF layout) + d^2 (fp32) from alpha."""
    a = alpha[0].astype(np.float64)  # [C]
    d = 1.0 - a
    # bf16 diag entries; d16 = 1 - a16 in bf16 arithmetic so the u_0 column
    # fixup (a16 + d16) lands as close to exactly 1 as bf16 allows
    a16 = a.astype(_NP_BF16)
    d16 = (_NP_BF16(1.0) - a16).astype(_NP_BF16)
    ad16 = (a16 * d16).astype(_NP_BF16)
    w = np.zeros((P, N_CB * 3 * P), dtype=_NP_BF16)
    idx = np.arange(P)
    for cb in range(N_CB):
        s = slice(cb * P, (cb + 1) * P)
        for j, v in enumerate((a16[s], ad16[s], d16[s])):
            w[idx, (cb * 3 + j) * P + idx] = v
    # d2 pre-arranged to the device SBUF layout [P, N_CB] (column j =
    # channel block j), so the load is one contiguous DMA
    d2 = np.ascontiguousarray(
        (d * d).astype(np.float32).reshape(N_CB, P).T
    )
    return w, d2


def _pack_core(xc: np.ndarray) -> np.ndarray:
    """[B_SH, C, L] -> [P, N_TILES*L] bf16: tile n = (cb*B_SH + b), block
    layout [evens | odds]."""
    x5 = xc.reshape(B_SH, N_CB, P, LH, 2)        # b, cb, p, m, parity
    x5 = x5.transpose(2, 1, 0, 4, 3)             # p, cb, b, parity, m
    return np.ascontiguousarray(
        x5.astype(_NP_BF16)
    ).reshape(P, N_TILES * L)


def _unpack_core(yc: np.ndarray) -> np.ndarray:
    """Inverse of _pack_core (+ the last tile's quarter-interleaved block)."""
    yc = yc.reshape(P, N_TILES, L).astype(np.float32)
    std = np.empty((P, N_TILES, 2, LH), dtype=np.float32)
    std[:, :, 0, :] = yc[:, :, 0:LH]
    std[:, :, 1, :] = yc[:, :, LH:L]
    lastq = yc[:, LAST].reshape(P, 4, 2, LQ)     # p, quarter, parity, m
    std[:, LAST, 0] = lastq[:, :, 0].reshape(P, LH)
    std[:, LAST, 1] = lastq[:, :, 1].reshape(P, LH)
    # std: p, (cb b), parity, m  ->  b, cb*P+p, 2m+parity
    y5 = std.reshape(P, N_CB, B_SH, 2, LH).transpose(2, 1, 0, 4, 3)
    return np.ascontiguousarray(y5).reshape(B_SH, C, L)


_cached_nc = None


def _get_nc() -> bass.Bass:
    global _cached_nc
    if _cached_nc is None:
        _cached_nc = build_nc()
    return _cached_nc


def run(x: np.ndarray, alpha: np.ndarray, nc=None, **spmd_kwargs):
    """Full host path: prep inputs, run on 8 cores, reassemble output.
    Returns (y, BassKernelResults)."""
    assert x.shape == (B, C, L) and alpha.shape == (1, C)
    x = np.asarray(x, dtype=np.float32)
    alpha = np.ascontiguousarray(alpha, dtype=np.float32)
    w, d2 = _host_consts(alpha)
    if nc is None:
        nc = _get_nc()
    in_maps = [
        {"x": _pack_core(x[c * B_SH : (c + 1) * B_SH]), "w": w, "d2": d2}
        for c in range(N_CORES)
    ]
    res = run_bass_kernel_spmd(nc, in_maps, list(range(N_CORES)), **spmd_kwargs)
    y = np.concatenate([_unpack_core(r["y"]) for r in res.results], axis=0)
    return y, res


def kernel(x: np.ndarray, alpha: np.ndarray) -> np.ndarray:
    return run(x, alpha)[0]
